# revision 2
# baseline (speedup 1.0000x reference)
"""Differential quadratic causal linear attention on 8 TRN2 NeuronCores.

Chunked-scan formulation (mathematically identical to the quadratic
reference): with augmented feature maps Q12 = [q1; -a*sigr(q2r)*q1] and
K12 = [k1; sigr(k2r)*k1] (f = 128 feature rows, sigr(x) = sig(relu(x))
= max(sig(x), 0.5)), causal attention is

  out[l] = intra-chunk masked part + Q12[:,l] . S_{c-1},
  S_c = S_{c-1} + K12[:,chunk c] @ v_aug[chunk c]    ([128, 65] state)

so per 128-wide chunk the PE does O(L*128) work instead of O(L^2/2).
Numerics: the feature maps run at f32r (their SIGNS decide which rows
are exactly zero; bf16 there resurrects zero rows with O(1) error);
everything after the relu is bf16 (zeros stay zero, no f32r small-N
matmul penalty, half the DMA/evac traffic, no power throttling).

Shapes (hardcoded): B=4, H=16, L=1024, D=64. Sharding: batch*heads
across 8 cores -> 8 (b,h) pairs per core; params sharded along H.
Host prep: q/k pre-transposed+stacked ([qT;kT] [128,L] per pair, f32),
v augmented with a ones column (bf16), weights stacked [W1|W2].

Per-pair pipeline (software-pipelined 3 deep, p scans while p+2 fmaps):
  fmap: [x1raw;x2raw] = W12^T @ qkT-half  (psum f32)
        top = relu(x1raw); bot = svec * max(max(sig(x2raw),.5)*x1raw, 0)
  scan_a: k12n = PE-transpose(K12T); per chunk pT = K12T_c^T @ Q12_c,
        masked-evac bf16; S chain via PSUM accumulate + per-chunk evac
  scan_b: o_nat[l,65] = pm_c^T @ v_c + Q12_c^T @ S_{c-1}  (natural
        orientation -> no output transposes); out = o/(denom+eps)
"""

import numpy as np

import concourse.bass as bass
import concourse.bacc as bacc
import concourse.mybir as mybir
import concourse.tile as tile
from concourse.bass_utils import run_bass_kernel_spmd

B, H, L, D = 4, 16, 1024, 64
NCORES = 8
HPC = H // NCORES          # heads per core
NP = B * HPC               # (b,h) pairs per core
NT = L // 128              # l-chunks of 128
EPS = 1e-6
F32 = mybir.dt.float32
BF16 = mybir.dt.bfloat16
AF = mybir.ActivationFunctionType
OP = mybir.AluOpType

_CACHE = {}


def _consts_np():
    """[128, 256] f32: identity | mask_u (keep m<=l)."""
    c = np.zeros((128, 256), dtype=np.float32)
    c[:, 0:128] = np.eye(128, dtype=np.float32)
    m = np.arange(128)[:, None]
    l = np.arange(128)[None, :]
    c[:, 128:256] = (m <= l).astype(np.float32)
    return c


def _build(alpha: float, reps: int = 1):
    nc = bacc.Bacc(trn_type="TRN2", target_bir_lowering=False, debug=False)

    F32R_ = mybir.dt.float32r
    # host supplies [qT; kT] stacked: [128, L] per pair, f32r
    qkt_d = nc.dram_tensor("qkt", [NP, 128, L], F32R_,
                           kind="ExternalInput").ap()
    v_d = nc.dram_tensor("v", [NP, L, D + 1], BF16, kind="ExternalInput").ap()
    # per-head stacked fmap weights: wq = [W1q | W2q] (f64 x 128), f32r:
    # feature-map signs decide which rows are exactly zero, so the fmap
    # runs at f32r; bf16 only after relu (zeros stay zero).
    F32R = mybir.dt.float32r
    wq_d = nc.dram_tensor("wq", [HPC, D, 2 * D], F32R, kind="ExternalInput").ap()
    wk_d = nc.dram_tensor("wk", [HPC, D, 2 * D], F32R, kind="ExternalInput").ap()
    out_d = nc.dram_tensor("out", [NP, L, D], BF16, kind="ExternalOutput").ap()

    cst_d = nc.inline_tensor(_consts_np(), name="consts").ap()

    with tile.TileContext(nc) as tc:
        with (
            tc.tile_pool(name="statics", bufs=1) as statics,
            tc.tile_pool(name="io", bufs=4) as io,
            tc.tile_pool(name="sb", bufs=4) as sb,
            tc.tile_pool(name="sc", bufs=4) as sc,
            tc.tile_pool(name="ssb", bufs=2) as ssb,
            tc.tile_pool(name="ps_fm", bufs=3, space="PSUM") as ps_fm,
            tc.tile_pool(name="ps_p", bufs=2, space="PSUM") as ps_p,
            tc.tile_pool(name="ps_o", bufs=2, space="PSUM") as ps_o,
            tc.tile_pool(name="ps_g", bufs=1, space="PSUM") as ps_g,
        ):
            cst = statics.tile([128, 256], F32, tag="cst")
            nc.sync.dma_start(out=cst, in_=cst_d)
            ident = cst[:, 0:128]
            mask_u = cst[:, 128:256]
            cstb = statics.tile([128, 128], BF16, tag="cstb")
            nc.vector.tensor_copy(cstb, cst[:, 0:128])
            ident_b = cstb[:, 0:128]

            # greedy DVE/ACT load balancer for PSUM->SBUF evacuations
            load = {"dve": 0.0, "act": 0.0}

            def evac_copy(dst, src, cols, relu=False):
                cd = cols * 0.55 + 120.0 + load["dve"]
                ca = cols * 1.0 + 290.0 + load["act"]
                if cd <= ca:
                    load["dve"] = cd
                    if relu:
                        nc.vector.tensor_relu(dst, src)
                    else:
                        nc.vector.tensor_copy(dst, src)
                else:
                    load["act"] = ca
                    if relu:
                        nc.scalar.activation(dst, src, AF.Relu)
                    else:
                        nc.scalar.copy(dst, src)

            # per-head stationary weight tiles
            wq_t, wk_t = [], []
            for hl in range(HPC):
                t_wq = statics.tile([64, 128], F32R, tag=f"wq{hl}",
                                    name=f"wq{hl}")
                nc.sync.dma_start(out=t_wq, in_=wq_d[hl])
                wq_t.append(t_wq)
                t_wk = statics.tile([128, 128], F32R, tag=f"wk{hl}",
                                    name=f"wk{hl}")
                nc.sync.dma_start(out=t_wk[64:128, :], in_=wk_d[hl])
                wk_t.append(t_wk)

            st = {}  # per-pair live tiles

            def stage_load(p):
                pd = p % NP
                qkT = sb.tile([128, L], F32R, tag="qkT", name=f"qkT{p}")
                for g in range(2):
                    cols = slice(g * 512, (g + 1) * 512)
                    nc.sync.dma_start(out=qkT[:, cols], in_=qkt_d[pd][:, cols])
                vn = io.tile([128, NT, 65], BF16, tag="vn", name=f"vn{p}")
                nc.sync.dma_start(
                    out=vn,
                    in_=v_d[pd].rearrange("(t pp) d -> pp t d", pp=128),
                )
                st[p] = {"qkT": qkT, "vn": vn}

            def stage_fmap(p):
                hl = p % HPC
                qkT = st[p]["qkT"]

                # --- transposed feature maps + epilogues ---
                # Aq = Q12 [f=128, l]; Kt = K12T [f=128, m]
                Aq = sb.tile([128, L], BF16, tag="Aq", name=f"Aq{p}")
                Kt = sb.tile([128, L], BF16, tag="Kt", name=f"Kt{p}")
                for qk in (1, 0):
                    dst = Aq if qk == 0 else Kt
                    wm = wq_t[hl] if qk == 0 else wk_t[hl][64:128, :]
                    pb = 0 if qk == 0 else 64
                    for lc in range(2):
                        cols = slice(lc * 512, (lc + 1) * 512)
                        sig = sc.tile([128, 512], F32, tag="sig",
                                      name=f"sig{p}_{qk}_{lc}")
                        tmp = sc.tile([128, 512], BF16, tag="tmp",
                                      name=f"tmp{p}_{qk}_{lc}")
                        fm = ps_fm.tile([128, 512], F32, tag="fm",
                                        name=f"fm{p}_{qk}_{lc}")
                        nc.tensor.matmul(
                            fm, wm, qkT[pb:pb + 64, cols],
                            start=True, stop=True,
                        )
                        # top half: x1 = relu(raw1)
                        nc.scalar.activation(dst[0:64, cols], fm[0:64, :],
                                             AF.Relu)
                        # sigma = sigmoid(raw2)
                        nc.scalar.activation(sig[64:128, :], fm[64:128, :],
                                             AF.Sigmoid)
                        # sig(relu(raw2)) = max(sig(raw2), 0.5); then
                        # sigr*relu(raw1) = max/min of sigr*raw1 vs 0
                        nc.vector.scalar_tensor_tensor(
                            tmp[64:128, :], sig[64:128, :], 0.5,
                            fm[0:64, :], op0=OP.max, op1=OP.mult,
                        )
                        if qk == 0:
                            # -a*sigr*relu(raw1) = max(tmp,0)*(-a)
                            nc.vector.tensor_scalar(
                                dst[64:128, cols], tmp[64:128, :],
                                0.0, -alpha, op0=OP.max, op1=OP.mult,
                            )
                        else:
                            # sigr*relu(raw1) = relu(tmp)
                            evac_copy(dst[64:128, cols], tmp[64:128, :],
                                      512, relu=True)

                st[p].update(Aq=Aq, Kt=Kt)

            def stage_scan_a(p):
                Aq = st[p]["Aq"]
                Kt = st[p]["Kt"]
                vn = st[p]["vn"]

                # k12 natural [m, f] via PE-transpose of K12T; emitted here
                # (not in fmap) so the PE queue never waits on the same
                # pair's freshly-built Kt
                k12n = sb.tile([128, L], BF16, tag="k12n", name=f"k12n{p}")
                for lc in range(2):
                    ktr = ps_fm.tile([128, 512], BF16, tag="fm",
                                     name=f"ktr{p}_{lc}")
                    for j in range(4):
                        c = lc * 4 + j
                        nc.tensor.transpose(
                            ktr[:, j * 128:(j + 1) * 128],
                            Kt[:, c * 128:(c + 1) * 128],
                            ident_b,
                        )
                    evac_copy(k12n[:, lc * 512:(lc + 1) * 512], ktr, 512)


                # --- S chain + pT blocks ---
                S_ps = ps_g.tile([128, 65], F32, tag="S", name=f"S{p}")
                S_sb = []
                pmask = []
                mu = mask_u[:, :]
                for half in range(2):
                    pp = ps_p.tile([128, 512], F32, tag="pp",
                                   name=f"pp{p}_{half}")
                    pm = ssb.tile([128, 4, 128], BF16, tag="pm",
                                  name=f"pm{p}_{half}")
                    j0 = 0
                    for j in range(j0, 4):
                        c = half * 4 + j
                        # pT block [m, l] for chunk c
                        nc.tensor.matmul(
                            pp[:, j * 128:(j + 1) * 128],
                            Kt[:, c * 128:(c + 1) * 128],
                            Aq[:, c * 128:(c + 1) * 128],
                            start=True, stop=True,
                        )
                        # S chain: G += k12n_c^T @ v_c, evac after each
                        if c < NT - 1:
                            nc.tensor.matmul(
                                S_ps,
                                k12n[:, c * 128:(c + 1) * 128],
                                vn[:, c, :],
                                start=(c == 0), stop=(c == NT - 2),
                                skip_group_check=True,
                            )
                            s_sb = ssb.tile([128, 65], BF16, tag="S_sb",
                                            bufs=8, name=f"Ssb{p}_{c}")
                            evac_copy(s_sb, S_ps, 65)
                            S_sb.append(s_sb)
                    # masked evac of chunks at once; mask bcast over dim 1
                    nj = 4 - j0
                    mu_b = bass.AP(tensor=mu.tensor, offset=mu.offset,
                                   ap=[list(mu.ap[0]), [0, nj],
                                       list(mu.ap[1])])
                    pp_ap = pp[:, :]
                    pp3 = bass.AP(tensor=pp_ap.tensor,
                                  offset=pp_ap.offset + j0 * 128,
                                  ap=[list(pp_ap.ap[0]), [128, nj],
                                      [1, 128]])
                    nc.vector.tensor_tensor(pm[:, j0:4, :], pp3, mu_b,
                                            op=OP.mult)
                    pmask.append(pm)
                st[p].update(S_sb=S_sb, pmask=pmask)

            def stage_scan_b(p):
                Aq = st[p]["Aq"]
                vn = st[p]["vn"]
                S_sb = st[p]["S_sb"]
                pmask = st[p]["pmask"]
                # --- o accumulation in natural [l, d] orientation ---
                outf = io.tile([128, NT, 64], BF16, tag="outf", name=f"outf{p}")
                for half in range(2):
                    on = ps_o.tile([128, 4, 128], F32, tag="on",
                                   name=f"on{p}_{half}")
                    for j in range(4):
                        c = half * 4 + j
                        # intra: w = masked pT chunk (stationary), rhs = v
                        nc.tensor.matmul(
                            on[:, j, 0:65],
                            pmask[half][:, j, :],
                            vn[:, c, :],
                            start=True, stop=(c == 0),
                            skip_group_check=True,
                        )
                        # inter: w = Q12 chunk, rhs = S_sb[c-1]
                        if c > 0:
                            nc.tensor.matmul(
                                on[:, j, 0:65],
                                Aq[:, c * 128:(c + 1) * 128],
                                S_sb[c - 1],
                                start=False, stop=True,
                                skip_group_check=True,
                            )
                    dc = sc.tile([128, 4], F32, tag="dc", name=f"dc{p}_{half}")
                    nc.vector.tensor_scalar(dc, on[:, :, 64], EPS, None,
                                            op0=OP.add)
                    di = sc.tile([128, 4], F32, tag="di", name=f"di{p}_{half}")
                    nc.vector.reciprocal(di, dc)
                    di_ap = di[:, :]
                    di_b = bass.AP(tensor=di_ap.tensor, offset=di_ap.offset,
                                   ap=list(di_ap.ap) + [[0, 64]])
                    nc.vector.tensor_tensor(
                        outf[:, half * 4:(half + 1) * 4, :],
                        on[:, :, 0:64], di_b, op=OP.mult,
                    )

                nc.sync.dma_start(
                    out=out_d[p % NP].rearrange("(t pp) d -> pp t d", pp=128),
                    in_=outf,
                )
                del st[p]

            # software pipeline: scan phase of pair p issues after the
            # fmap phase of pair p+1
            seq = list(range(reps * NP))
            stage_load(seq[0])
            stage_fmap(seq[0])
            if len(seq) > 1:
                stage_load(seq[1])
                stage_fmap(seq[1])
            if len(seq) > 2:
                stage_load(seq[2])
            for i, p in enumerate(seq):
                if i + 3 < len(seq):
                    stage_load(seq[i + 3])
                stage_scan_a(p)
                if i + 2 < len(seq):
                    stage_fmap(seq[i + 2])
                stage_scan_b(p)
    nc.compile()
    return nc


def _get_nc(alpha: float = 0.3, reps: int = 1):
    key = ("nc", float(alpha), reps)
    if key not in _CACHE:
        _CACHE[key] = _build(float(alpha), reps)
    return _CACHE[key]


def _prep_core_inputs(q, k, v, W1q, W1k, W2q, W2k):
    """Host-side shard prep: bf16 casts + weight stacking, per core."""
    import ml_dtypes
    bf = ml_dtypes.bfloat16
    v_aug = np.concatenate(
        [v, np.ones(v.shape[:-1] + (1,), np.float32)], axis=-1)
    wq = np.concatenate([W1q, W2q], axis=-1)   # [H, D, 2D]
    wk = np.concatenate([W1k, W2k], axis=-1)
    # [qT; kT] stacked per pair: [NP, 128, L]
    qkt = np.concatenate([
        q.transpose(0, 1, 3, 2), k.transpose(0, 1, 3, 2)], axis=2)
    in_maps = []
    for c in range(NCORES):
        hs = slice(c * HPC, (c + 1) * HPC)
        in_maps.append({
            "qkt": np.ascontiguousarray(qkt[:, hs].reshape(NP, 128, L)),
            "v": np.ascontiguousarray(
                v_aug[:, hs].reshape(NP, L, D + 1).astype(bf)),

            "wq": np.ascontiguousarray(wq[hs]),
            "wk": np.ascontiguousarray(wk[hs]),
        })
    return in_maps


def kernel(query_states, key_states, value_states, W1q, W1k, W2q, W2k, alpha):
    q = np.ascontiguousarray(np.asarray(query_states, dtype=np.float32))
    k = np.ascontiguousarray(np.asarray(key_states, dtype=np.float32))
    v = np.ascontiguousarray(np.asarray(value_states, dtype=np.float32))
    w1q = np.asarray(W1q, dtype=np.float32)
    w1k = np.asarray(W1k, dtype=np.float32)
    w2q = np.asarray(W2q, dtype=np.float32)
    w2k = np.asarray(W2k, dtype=np.float32)
    al = float(np.asarray(alpha, dtype=np.float32).reshape(-1)[0])

    nc = _get_nc(al)
    in_maps = _prep_core_inputs(q, k, v, w1q, w1k, w2q, w2k)
    res = run_bass_kernel_spmd(nc, in_maps, core_ids=list(range(NCORES)))
    out = np.empty((B, H, L, D), dtype=np.float32)
    for c in range(NCORES):
        o = res.results[c]["out"].reshape(B, HPC, L, D)
        out[:, c * HPC:(c + 1) * HPC] = o.astype(np.float32)
    return out
